# revision 5
# baseline (speedup 1.0000x reference)
"""DistMult scoring kernel for Trainium2 (8 NeuronCores, Bass/Tile).

reference computation:
    rel = rel_embeds[rel_ids]                      # [B, D] gather
    scores = sum(head * rel * tail, axis=-1)       # [B]
    pos = min(scores[:n_pos], upper_bound)
    neg = max(scores[n_pos:], lower_bound)
    out = sigmoid(concat(pos, neg))

Sharding: data-parallel over B. Core c owns rows [c*65536, (c+1)*65536).
Within a core, local row r maps to (partition p, column t) with r = p*512 + t,
which makes every stream DMA contiguous per partition and lets the final
[128, 512] score tile be stored with a single contiguous DMA.

The pos/neg split falls on a core boundary (131072 = 2 * 65536), handled
data-parallel by feeding cores +/-inf padded bounds:
    out = sigmoid(max(min(scores, ub), lb))
with ub=+inf for neg cores and lb=-inf for pos cores.
"""

import sys

for _p in ("/opt/trn_rl_repo",):
    if _p not in sys.path:
        sys.path.insert(0, _p)

import numpy as np

import concourse.bacc as bacc
import concourse.bass as bass
import concourse.mybir as mybir
import concourse.tile as tile
from concourse.bass_utils import run_bass_kernel_spmd

N_POS = 131072
N_NEG = 393216
B = N_POS + N_NEG  # 524288
D = 256
NUM_REL = 500
NCORES = 8
ROWS = B // NCORES  # 65536 rows per core
P = 128
T = ROWS // P  # 512 tiles of 128 rows; local row = p*T + t
GROUP = 8  # tiles per loop iteration
NG = T // GROUP  # 64 iterations

# stream dtype for head/tail/rel table ("f32" or "bf16")
STREAM_DT = "f32"


def build_program(stream_dt: str = STREAM_DT):
    sdt = mybir.dt.float32 if stream_dt == "f32" else mybir.dt.bfloat16
    f32 = mybir.dt.float32
    i32 = mybir.dt.int32
    mult = mybir.AluOpType.mult
    add = mybir.AluOpType.add

    nc = bacc.Bacc(
        "TRN2", target_bir_lowering=False, debug=False, num_devices=NCORES
    )
    h = nc.declare_dram_parameter("h", [ROWS, D], sdt, isOutput=False)
    t_ = nc.declare_dram_parameter("t", [ROWS, D], sdt, isOutput=False)
    ids = nc.declare_dram_parameter("ids", [ROWS], i32, isOutput=False)
    ub = nc.declare_dram_parameter("ub", [ROWS], f32, isOutput=False)
    lb = nc.declare_dram_parameter("lb", [ROWS], f32, isOutput=False)
    table = nc.declare_dram_parameter("table", [NUM_REL, D], sdt, isOutput=False)
    out = nc.declare_dram_parameter("out", [ROWS], f32, isOutput=True)

    h_v = h[:].rearrange("(p t) d -> p t d", p=P)
    t_v = t_[:].rearrange("(p t) d -> p t d", p=P)
    ids_v = ids[:].rearrange("(p t) -> p t", p=P)
    ub_v = ub[:].rearrange("(p t) -> p t", p=P)
    lb_v = lb[:].rearrange("(p t) -> p t", p=P)
    out_v = out[:].rearrange("(p t) -> p t", p=P)

    with tile.TileContext(nc) as tc:
        with (
            tc.tile_pool(name="io", bufs=1) as io_pool,
            tc.tile_pool(name="stream", bufs=3) as spool,
            tc.tile_pool(name="scratch", bufs=2) as qpool,
        ):
            ids_all = io_pool.tile([P, T], i32)
            nc.sync.dma_start(out=ids_all[:], in_=ids_v)
            scores = io_pool.tile([P, T], f32)

            for g in range(NG):
                t0 = g * GROUP
                htile = spool.tile([P, GROUP * D], sdt, tag="h")
                ttile = spool.tile([P, GROUP * D], sdt, tag="t")
                rtile = spool.tile([P, GROUP * D], sdt, tag="r")
                nc.sync.dma_start(
                    out=htile[:].rearrange("p (g d) -> p g d", g=GROUP),
                    in_=h_v[:, t0 : t0 + GROUP, :],
                )
                nc.sync.dma_start(
                    out=ttile[:].rearrange("p (g d) -> p g d", g=GROUP),
                    in_=t_v[:, t0 : t0 + GROUP, :],
                )
                for gi in range(GROUP):
                    nc.gpsimd.indirect_dma_start(
                        out=rtile[:, gi * D : (gi + 1) * D],
                        out_offset=None,
                        in_=table[:],
                        in_offset=bass.IndirectOffsetOnAxis(
                            ap=ids_all[:, t0 + gi : t0 + gi + 1], axis=0
                        ),
                    )
                q = qpool.tile([P, GROUP * D], sdt, tag="q")
                s = qpool.tile([P, GROUP * D], sdt, tag="s")
                nc.vector.tensor_tensor(
                    out=q[:], in0=htile[:], in1=ttile[:], op=mult
                )
                for gi in range(GROUP):
                    seg = slice(gi * D, (gi + 1) * D)
                    nc.vector.scalar_tensor_tensor(
                        out=s[:, seg],
                        in0=q[:, seg],
                        scalar=1.0,
                        in1=rtile[:, seg],
                        op0=mult,
                        op1=mult,
                        accum_out=scores[:, t0 + gi : t0 + gi + 1],
                    )

            # tail: clamp + sigmoid + store
            ubt = io_pool.tile([P, T], f32)
            lbt = io_pool.tile([P, T], f32)
            nc.sync.dma_start(out=ubt[:], in_=ub_v)
            nc.sync.dma_start(out=lbt[:], in_=lb_v)
            clip1 = io_pool.tile([P, T], f32)
            clip2 = io_pool.tile([P, T], f32)
            nc.vector.tensor_tensor(
                out=clip1[:], in0=scores[:], in1=ubt[:], op=mybir.AluOpType.min
            )
            nc.vector.tensor_tensor(
                out=clip2[:], in0=clip1[:], in1=lbt[:], op=mybir.AluOpType.max
            )
            sig = io_pool.tile([P, T], f32)
            nc.scalar.activation(
                out=sig[:], in_=clip2[:], func=mybir.ActivationFunctionType.Sigmoid
            )
            nc.sync.dma_start(out=out_v, in_=sig[:])

    nc.compile()
    return nc


def make_in_maps(inputs: dict, stream_dt: str = STREAM_DT):
    np_sdt = np.float32 if stream_dt == "f32" else None
    import ml_dtypes

    if np_sdt is None:
        np_sdt = ml_dtypes.bfloat16

    head = np.asarray(inputs["head_embeds"], dtype=np.float32)
    tail = np.asarray(inputs["tail_embeds"], dtype=np.float32)
    rel_ids = np.asarray(inputs["rel_ids"]).astype(np.int32)
    lower = np.asarray(inputs["lower_bound"], dtype=np.float32)
    upper = np.asarray(inputs["upper_bound"], dtype=np.float32)
    table = np.asarray(inputs["rel_embeds"], dtype=np.float32).astype(np_sdt)

    head = head.astype(np_sdt)
    tail = tail.astype(np_sdt)

    pos_inf = np.full(ROWS, np.inf, dtype=np.float32)
    neg_inf = np.full(ROWS, -np.inf, dtype=np.float32)

    in_maps = []
    for c in range(NCORES):
        lo = c * ROWS
        hi = lo + ROWS
        if hi <= N_POS:
            ub_c = upper[lo:hi]
            lb_c = neg_inf
        else:
            assert lo >= N_POS
            ub_c = pos_inf
            lb_c = lower[lo - N_POS : hi - N_POS]
        in_maps.append(
            {
                "h": np.ascontiguousarray(head[lo:hi]),
                "t": np.ascontiguousarray(tail[lo:hi]),
                "ids": np.ascontiguousarray(rel_ids[lo:hi]),
                "ub": np.ascontiguousarray(ub_c),
                "lb": np.ascontiguousarray(lb_c),
                "table": table,
            }
        )
    return in_maps


def kernel(**inputs) -> np.ndarray:
    nc = build_program(STREAM_DT)
    in_maps = make_in_maps(inputs, STREAM_DT)
    res = run_bass_kernel_spmd(nc, in_maps, list(range(NCORES)))
    return np.concatenate([res.results[c]["out"] for c in range(NCORES)])


# revision 6
# speedup vs baseline: 1.0952x; 1.0952x over previous
"""DistMult scoring kernel for Trainium2 (8 NeuronCores, Bass/Tile).

reference computation:
    rel = rel_embeds[rel_ids]                      # [B, D] gather
    scores = sum(head * rel * tail, axis=-1)       # [B]
    pos = min(scores[:n_pos], upper_bound)
    neg = max(scores[n_pos:], lower_bound)
    out = sigmoid(concat(pos, neg))

Sharding: data-parallel over B. Core c owns rows [c*65536, (c+1)*65536).
Within a core, local row r maps to (partition p, column t) with r = p*512 + t,
which makes every stream DMA contiguous per partition and lets the final
[128, 512] score tile be stored with a single contiguous DMA.

The pos/neg split falls on a core boundary (131072 = 2 * 65536), handled
data-parallel by feeding cores +/-inf padded bounds:
    out = sigmoid(max(min(scores, ub), lb))
with ub=+inf for neg cores and lb=-inf for pos cores.
"""

import sys

for _p in ("/opt/trn_rl_repo",):
    if _p not in sys.path:
        sys.path.insert(0, _p)

import numpy as np

import concourse.bacc as bacc
import concourse.bass as bass
import concourse.mybir as mybir
import concourse.tile as tile
from concourse.bass_utils import run_bass_kernel_spmd

N_POS = 131072
N_NEG = 393216
B = N_POS + N_NEG  # 524288
D = 256
NUM_REL = 500
NCORES = 8
ROWS = B // NCORES  # 65536 rows per core
P = 128
T = ROWS // P  # 512 tiles of 128 rows; local row = p*T + t
GROUP = 8  # tiles per loop iteration
NG = T // GROUP  # 64 iterations

# stream dtype for head/tail/rel table ("f32" or "bf16")
STREAM_DT = "bf16"


def build_program(stream_dt: str = STREAM_DT):
    sdt = mybir.dt.float32 if stream_dt == "f32" else mybir.dt.bfloat16
    f32 = mybir.dt.float32
    i32 = mybir.dt.int32
    mult = mybir.AluOpType.mult
    add = mybir.AluOpType.add

    nc = bacc.Bacc(
        "TRN2", target_bir_lowering=False, debug=False, num_devices=NCORES
    )
    h = nc.declare_dram_parameter("h", [ROWS, D], sdt, isOutput=False)
    t_ = nc.declare_dram_parameter("t", [ROWS, D], sdt, isOutput=False)
    ids = nc.declare_dram_parameter("ids", [ROWS], i32, isOutput=False)
    ub = nc.declare_dram_parameter("ub", [ROWS], f32, isOutput=False)
    lb = nc.declare_dram_parameter("lb", [ROWS], f32, isOutput=False)
    table = nc.declare_dram_parameter("table", [NUM_REL, D], sdt, isOutput=False)
    out = nc.declare_dram_parameter("out", [ROWS], f32, isOutput=True)

    h_v = h[:].rearrange("(p t) d -> p t d", p=P)
    t_v = t_[:].rearrange("(p t) d -> p t d", p=P)
    ids_v = ids[:].rearrange("(p t) -> p t", p=P)
    ub_v = ub[:].rearrange("(p t) -> p t", p=P)
    lb_v = lb[:].rearrange("(p t) -> p t", p=P)
    out_v = out[:].rearrange("(p t) -> p t", p=P)

    with tile.TileContext(nc) as tc:
        with (
            tc.tile_pool(name="io", bufs=1) as io_pool,
            tc.tile_pool(name="stream", bufs=3) as spool,
            tc.tile_pool(name="scratch", bufs=2) as qpool,
        ):
            ids_all = io_pool.tile([P, T], i32)
            nc.sync.dma_start(out=ids_all[:], in_=ids_v)
            scores = io_pool.tile([P, T], f32)

            for g in range(NG):
                t0 = g * GROUP
                htile = spool.tile([P, GROUP * D], sdt, tag="h")
                ttile = spool.tile([P, GROUP * D], sdt, tag="t")
                rtile = spool.tile([P, GROUP * D], sdt, tag="r")
                nc.sync.dma_start(
                    out=htile[:].rearrange("p (g d) -> p g d", g=GROUP),
                    in_=h_v[:, t0 : t0 + GROUP, :],
                )
                nc.sync.dma_start(
                    out=ttile[:].rearrange("p (g d) -> p g d", g=GROUP),
                    in_=t_v[:, t0 : t0 + GROUP, :],
                )
                for gi in range(GROUP):
                    nc.gpsimd.indirect_dma_start(
                        out=rtile[:, gi * D : (gi + 1) * D],
                        out_offset=None,
                        in_=table[:],
                        in_offset=bass.IndirectOffsetOnAxis(
                            ap=ids_all[:, t0 + gi : t0 + gi + 1], axis=0
                        ),
                    )
                q = qpool.tile([P, GROUP * D], sdt, tag="q")
                s = qpool.tile([P, GROUP * D], sdt, tag="s")
                nc.vector.tensor_tensor(
                    out=q[:], in0=htile[:], in1=ttile[:], op=mult
                )
                for gi in range(GROUP):
                    seg = slice(gi * D, (gi + 1) * D)
                    nc.vector.scalar_tensor_tensor(
                        out=s[:, seg],
                        in0=q[:, seg],
                        scalar=1.0,
                        in1=rtile[:, seg],
                        op0=mult,
                        op1=mult,
                        accum_out=scores[:, t0 + gi : t0 + gi + 1],
                    )

            # tail: clamp + sigmoid + store
            ubt = io_pool.tile([P, T], f32)
            lbt = io_pool.tile([P, T], f32)
            nc.sync.dma_start(out=ubt[:], in_=ub_v)
            nc.sync.dma_start(out=lbt[:], in_=lb_v)
            clip1 = io_pool.tile([P, T], f32)
            clip2 = io_pool.tile([P, T], f32)
            nc.vector.tensor_tensor(
                out=clip1[:], in0=scores[:], in1=ubt[:], op=mybir.AluOpType.min
            )
            nc.vector.tensor_tensor(
                out=clip2[:], in0=clip1[:], in1=lbt[:], op=mybir.AluOpType.max
            )
            sig = io_pool.tile([P, T], f32)
            nc.scalar.activation(
                out=sig[:], in_=clip2[:], func=mybir.ActivationFunctionType.Sigmoid
            )
            nc.sync.dma_start(out=out_v, in_=sig[:])

    nc.compile()
    return nc


def make_in_maps(inputs: dict, stream_dt: str = STREAM_DT):
    np_sdt = np.float32 if stream_dt == "f32" else None
    import ml_dtypes

    if np_sdt is None:
        np_sdt = ml_dtypes.bfloat16

    head = np.asarray(inputs["head_embeds"], dtype=np.float32)
    tail = np.asarray(inputs["tail_embeds"], dtype=np.float32)
    rel_ids = np.asarray(inputs["rel_ids"]).astype(np.int32)
    lower = np.asarray(inputs["lower_bound"], dtype=np.float32)
    upper = np.asarray(inputs["upper_bound"], dtype=np.float32)
    table = np.asarray(inputs["rel_embeds"], dtype=np.float32).astype(np_sdt)

    head = head.astype(np_sdt)
    tail = tail.astype(np_sdt)

    pos_inf = np.full(ROWS, np.inf, dtype=np.float32)
    neg_inf = np.full(ROWS, -np.inf, dtype=np.float32)

    in_maps = []
    for c in range(NCORES):
        lo = c * ROWS
        hi = lo + ROWS
        if hi <= N_POS:
            ub_c = upper[lo:hi]
            lb_c = neg_inf
        else:
            assert lo >= N_POS
            ub_c = pos_inf
            lb_c = lower[lo - N_POS : hi - N_POS]
        in_maps.append(
            {
                "h": np.ascontiguousarray(head[lo:hi]),
                "t": np.ascontiguousarray(tail[lo:hi]),
                "ids": np.ascontiguousarray(rel_ids[lo:hi]),
                "ub": np.ascontiguousarray(ub_c),
                "lb": np.ascontiguousarray(lb_c),
                "table": table,
            }
        )
    return in_maps


def kernel(**inputs) -> np.ndarray:
    nc = build_program(STREAM_DT)
    in_maps = make_in_maps(inputs, STREAM_DT)
    res = run_bass_kernel_spmd(nc, in_maps, list(range(NCORES)))
    return np.concatenate([res.results[c]["out"] for c in range(NCORES)])
